# revision 13
# baseline (speedup 1.0000x reference)
"""Multi-head causal attention (B=2, S=2048, D=1024, H=16, dh=64) on 8
Trainium2 NeuronCores.

Sharding: core i handles batch b = i//4 and head group g = i%4 (4 heads
each).  Per core everything is computed in a transposed layout:

  QT = Wq_g^T @ x_b^T          [256(hk), 2048(S)]   (bf16)
  KT = Wk_g^T @ x_b^T          [256(hk), 2048(S)]   (bf16)
  V  = x_b @ Wv_g              [2048(S), 4, 65]     (bf16; col 64 = ones)
  per chunk c (512 queries), head-pair hp, key block j (128 keys):
     scT[par] = KT_h[:,j]^T(lhsT) x QT_h[:,c]   -> PSUM [128, 2, 512]
                (the two heads of a pair use PE rows 0-63 / 64-127 and
                 run concurrently)
     expT     = exp(scT/8) (* causal mask when j >= 4c)        (bf16)
     zT_h    += V_aug[j]^T(lhsT) x expT[par]    -> PSUM [65, 512]
                (row 64 accumulates the softmax denominator s)
     ztn      = zT[0:64] * broadcast(1/s)       [256(hk), 2048] (bf16)
  outT(c) = Wo_g^T(lhsT) x ztn(c)               [1024(d), 512]  (bf16)

v2 scheduling changes vs v1:
  - The scalar engine is the pacer of the attention stretch (exp runs at
    (N+352)/1.2 ns and is ACT-only), so ALL other ACT work was moved off
    it: Q/K PSUM drains (+bias) now run on DVE via tensor_scalar_add.
  - Input DMA is split across both HWDGE rings (sync + scalar) with the
    critical prefix first (wq/wk m0 halves, x chunk 0 per-di), so the
    first exp fires ~8us earlier.
  - phase_d(c) is emitted shift-1 (after attn(c+1,1)) so output
    projections backfill PE idle slots inside the ACT-paced stretch
    instead of forming a serial tail.  Chunks 0-2 collect into a
    per-chunk SBUF buffer shipped by one DMA; the last chunk drains
    per-dt with copies alternating ACT/DVE and DMAs alternating
    sync/scalar rings to shorten the drain.

Host: shards/transposes inputs, sums the 4 head-group partial outputs
per batch, adds b_O and the exact b_V fold (softmax rows sum to 1):
  out += b_O + sum_h b_V[h] @ W_O[h].
"""
import numpy as np
import ml_dtypes

import concourse.bacc as bacc
import concourse.mybir as mybir
import concourse.tile as tile
from concourse.bass_utils import run_bass_kernel_spmd

f32 = mybir.dt.float32
bf16 = mybir.dt.bfloat16
AF = mybir.ActivationFunctionType

B, S, D, H, DH = 2, 2048, 1024, 16, 64
NCORES = 8
HG = 4                # heads per core
HK = HG * DH          # 256
CH = 512              # query chunk
NCH = S // CH         # 4
KB = 128              # key block
DT = D // 128         # 8

_CACHE = {}


def _build_nc():
    nc = bacc.Bacc(None, target_bir_lowering=False, debug=False,
                   num_devices=NCORES)

    # x^T tiled chunk-major: [128, c, di, 512]
    xt_d = nc.dram_tensor("xt", [128, NCH, DT, CH], bf16,
                          kind="ExternalInput")
    wq_d = nc.dram_tensor("wq", [128, DT, HK], bf16, kind="ExternalInput")
    wk_d = nc.dram_tensor("wk", [128, DT, HK], bf16, kind="ExternalInput")
    wv_d = nc.dram_tensor("wv", [128, DT, HK], bf16, kind="ExternalInput")
    wo_d = nc.dram_tensor("wo", [128, 2, D], bf16, kind="ExternalInput")
    bq_d = nc.dram_tensor("bq", [128, 2], f32, kind="ExternalInput")
    bk_d = nc.dram_tensor("bk", [128, 2], f32, kind="ExternalInput")
    # relative causal mask for a 128-key diagonal band: [128, 2(par), 128]
    mask_d = nc.dram_tensor("mask", [128, 2, KB], bf16, kind="ExternalInput")
    out_d = nc.dram_tensor("outT", [128, DT, S], bf16, kind="ExternalOutput")

    with tile.TileContext(nc) as tc:
        with (
            tc.tile_pool(name="const", bufs=1) as cp,
            tc.tile_pool(name="big", bufs=1) as bp,
            tc.tile_pool(name="work", bufs=3) as wp,
            tc.tile_pool(name="psum", bufs=2, space="PSUM") as pp,
        ):
            wq = cp.tile([128, DT, HK], bf16)
            wk = cp.tile([128, DT, HK], bf16)
            wv = cp.tile([128, DT, HK], bf16)
            wo = cp.tile([128, 2, D], bf16)
            bq = cp.tile([128, 2], f32)
            bk = cp.tile([128, 2], f32)
            mask = cp.tile([128, 2, KB], bf16)
            xt = bp.tile([128, NCH, DT, CH], bf16)

            # ---- input DMA, split across both HWDGE rings plus the
            # SWDGE (gpsimd) ring.  HBM bandwidth is the binding
            # resource at startup, so the critical prefix for the first
            # projection chain (Wq/Wk m0 halves + ALL of x chunk 0) goes
            # first on BOTH rings; everything else strictly after.  Wv
            # rides HWDGE too (SWDGE is ~100GB/s and would deliver it
            # too late for the first AV matmul).
            nc.sync.dma_start(wq[:, :, 0:128], wq_d[:, :, 0:128])
            nc.scalar.dma_start(wk[:, :, 0:128], wk_d[:, :, 0:128])
            for p in range(2):
                nc.sync.dma_start(xt[:, 0, 2 * p:2 * p + 2, :],
                                  xt_d[:, 0, 2 * p:2 * p + 2, :])
                nc.scalar.dma_start(xt[:, 0, 4 + 2 * p:6 + 2 * p, :],
                                    xt_d[:, 0, 4 + 2 * p:6 + 2 * p, :])
            nc.sync.dma_start(xt[:, 1], xt_d[:, 1])
            nc.scalar.dma_start(wv, wv_d[:])
            nc.sync.dma_start(wq[:, :, 128:256], wq_d[:, :, 128:256])
            nc.scalar.dma_start(wk[:, :, 128:256], wk_d[:, :, 128:256])
            nc.sync.dma_start(xt[:, 2], xt_d[:, 2])
            nc.scalar.dma_start(xt[:, 3], xt_d[:, 3])
            nc.gpsimd.dma_start(bq, bq_d[:])
            nc.gpsimd.dma_start(bk, bk_d[:])
            nc.gpsimd.dma_start(mask, mask_d[:])
            nc.gpsimd.dma_start(wo, wo_d[:])

            qt = bp.tile([128, 2, S], bf16)
            kt = bp.tile([128, 2, S], bf16)
            # V padded to 128 columns (col 64 = ones for the softmax
            # denominator; 65.. zero) so the zT matmul's stationary is
            # 128-wide -> fast weight load
            v = bp.tile([128, S // KB, HG, 128], bf16)
            ztn = bp.tile([128, 2, S], bf16)

            # v padding (ones + zeros) on gpsimd: it is idle at startup
            # and this must NOT delay the DVE memsets that gate the
            # HAM warm-up matmuls.
            nc.gpsimd.memset(v[:, :, :, DH:DH + 1], 1.0)
            nc.gpsimd.memset(v[:, :, :, DH + 1:], 0.0)

            # preload the exp ACT table set at t=0 (one-time ~2.7us,
            # overlapped with input DMA)
            warm = wp.tile([1, 1], f32, tag="warm", bufs=1)
            nc.vector.memset(warm, 0.0)
            nc.scalar.activation(warm, warm, AF.Exp)

            # HAM warm-up: dummy matmuls from PE program start until the
            # first projections' DMA lands keep the PE clock gate open,
            # so real work runs at 2.4GHz from the first instruction.
            wrm = wp.tile([128, 128], bf16, tag="wrm", bufs=1)
            nc.vector.memset(wrm, 0.0)
            ps_w = pp.tile([128, KB], f32, tag="proj", name="ps_warm")
            for r in range(48):
                nc.tensor.matmul(ps_w, wrm, wrm, start=(r == 0),
                                 stop=(r == 47))

            # ---- backfill machinery.  The Tile scheduler builds static
            # in-order per-engine programs (priority = emission order
            # among data-ready ops), so projection / output-projection
            # work must be emitted in SMALL atoms interleaved between
            # attention blocks — otherwise it serializes at chunk
            # boundaries and starves the exp-paced scalar engine.
            backfill = []

            def pump():
                if backfill:
                    backfill.pop(0)()

            def chain_q(c, m):
                def th():
                    cs = c * CH
                    ps_q = pp.tile([128, CH], f32, tag="proj",
                                   name=f"ps_q_{c}_{m}")
                    for di in range(DT):
                        nc.tensor.matmul(
                            ps_q, wq[:, di, m * 128:(m + 1) * 128],
                            xt[:, c, di, :],
                            start=(di == 0), stop=(di == DT - 1))
                    nc.vector.tensor_scalar_add(
                        out=qt[:, m, cs:cs + CH], in0=ps_q,
                        scalar1=bq[:, m:m + 1])
                return th

            def chain_k(c, m):
                def th():
                    cs = c * CH
                    ps_k = pp.tile([128, CH], f32, tag="proj",
                                   name=f"ps_k_{c}_{m}")
                    for di in range(DT):
                        nc.tensor.matmul(
                            ps_k, wk[:, di, m * 128:(m + 1) * 128],
                            xt[:, c, di, :],
                            start=(di == 0), stop=(di == DT - 1))
                    nc.vector.tensor_scalar_add(
                        out=kt[:, m, cs:cs + CH], in0=ps_k,
                        scalar1=bk[:, m:m + 1])
                return th

            def chain_v(c, si):
                def th():
                    sl = (si - 4 * c) * KB
                    ps_v = pp.tile([128, HG, DH], f32, tag="proj",
                                   name=f"ps_v_{si}")
                    for di in range(DT):
                        nc.tensor.matmul(
                            ps_v, xt[:, c, di, sl:sl + KB], wv[:, di, :],
                            start=(di == 0), stop=(di == DT - 1))
                    nc.vector.tensor_copy(v[:, si, :, 0:DH], ps_v)
                return th

            ostc_tiles = {}

            def pd_group(c, dt_i):
                """One output-projection group (chunks 0..NCH-2): two
                accumulating matmuls + DVE drain into the per-chunk
                staging buffer; the last group ships the whole chunk
                with a single sync-ring DMA."""
                def th():
                    cs = c * CH
                    if c not in ostc_tiles:
                        ostc_tiles[c] = wp.tile(
                            [128, DT, CH], bf16, tag="ostc", bufs=2,
                            name=f"ostc_{c}")
                    ostc = ostc_tiles[c]
                    ps_o = pp.tile([128, CH], f32, tag="proj",
                                   name=f"ps_o_{c}_{dt_i}")
                    for m in range(2):
                        nc.tensor.matmul(
                            ps_o, wo[:, m, dt_i * 128:(dt_i + 1) * 128],
                            ztn[:, m, cs:cs + CH],
                            start=(m == 0), stop=(m == 1))
                    nc.vector.tensor_copy(ostc[:, dt_i, :], ps_o)
                    if dt_i == DT - 1:
                        nc.sync.dma_start(out_d[:, :, cs:cs + CH], ostc)
                return th

            def attn(c, hp):
                """Attention for (query chunk c, head pair hp)."""
                cs = c * CH
                m = hp
                nblk = 4 * c + 4
                zt0 = pp.tile([128, CH], f32, tag="zt0", bufs=1,
                              name=f"zt0_{c}_{hp}")
                zt1 = pp.tile([128, CH], f32, tag="zt1", bufs=1,
                              name=f"zt1_{c}_{hp}")
                zts = (zt0, zt1)
                for j in range(nblk):
                    pump()
                    # diagonal blocks (t>=0): queries below 128t are fully
                    # masked -> compute only [128t, CH)
                    t = j - 4 * c
                    ql = 128 * t if t > 0 else 0
                    sc = pp.tile([128, 2, CH], f32, tag="sc",
                                 name=f"sc_{c}_{hp}_{j}")
                    for par in range(2):
                        o = par * 64
                        nc.tensor.matmul(
                            sc[:, par, ql:],
                            kt[o:o + 64, m, j * KB:(j + 1) * KB],
                            qt[o:o + 64, m, cs + ql:cs + CH],
                            start=True, stop=True)
                    ex = wp.tile([128, 2, CH], bf16, tag="ex", bufs=6)
                    nc.scalar.activation(ex[:, :, ql:], sc[:, :, ql:],
                                         AF.Exp, scale=0.125)
                    if t >= 0:
                        qm = ql + 128
                        nc.vector.tensor_mul(ex[:, :, ql:qm],
                                             ex[:, :, ql:qm], mask)
                    for par in range(2):
                        h = 2 * hp + par
                        nc.tensor.matmul(
                            zts[par][:, ql:], v[:, j, h, :],
                            ex[:, par, ql:],
                            start=(j == 0), stop=(j == nblk - 1))
                # normalize: ztn[h] = zt[0:64] / zt[64].  First bounce the
                # zT+denominator block to SBUF so the PSUM accumulator
                # frees immediately for the next head pair; the divide
                # chain then runs from SBUF.  The very last pair has no
                # successor wanting its PSUM slot, so skip the bounce.
                last = (c == NCH - 1 and hp == 1)
                for par in range(2):
                    h = 2 * hp + par
                    o = par * 64
                    if last:
                        zsrc = zts[par]
                    else:
                        zs = wp.tile([DH + 1, CH], f32, tag="zs", bufs=3,
                                     name=f"zs_{c}_{h}")
                        nc.vector.tensor_copy(zs, zts[par][0:DH + 1, :])
                        zsrc = zs
                    srow = wp.tile([1, CH], f32, tag="srow", bufs=3,
                                   name=f"srow_{c}_{h}")
                    nc.vector.tensor_copy(srow, zsrc[DH:DH + 1, :])
                    rec = wp.tile([1, CH], f32, tag="rec", bufs=3,
                                  name=f"rec_{c}_{h}")
                    nc.vector.reciprocal_approx_fast(rec, srow)
                    bc = wp.tile([64, CH], f32, tag="bc", bufs=3,
                                 name=f"bc_{c}_{h}")
                    nc.gpsimd.partition_broadcast(bc, rec)
                    nc.vector.tensor_mul(ztn[o:o + 64, m, cs:cs + CH],
                                         zsrc[0:DH, :], bc)

            def phase_d_last():
                """Output projection for the last chunk: per-dt drain
                with copies alternating DVE/ACT (ACT is idle in the
                tail) and DMAs alternating sync/scalar rings."""
                c = NCH - 1
                cs = c * CH
                for dt_i in range(DT):
                    ps_o = pp.tile([128, CH], f32, tag="proj",
                                   name=f"ps_o_{c}_{dt_i}")
                    for m in range(2):
                        nc.tensor.matmul(
                            ps_o, wo[:, m, dt_i * 128:(dt_i + 1) * 128],
                            ztn[:, m, cs:cs + CH],
                            start=(m == 0), stop=(m == 1))
                    ost = wp.tile([128, CH], bf16, tag="ost", bufs=4,
                                  name=f"ost_{c}_{dt_i}")
                    if dt_i % 2 == 0:
                        nc.vector.tensor_copy(ost, ps_o)
                        nc.sync.dma_start(
                            out_d[:, dt_i, cs:cs + CH], ost)
                    else:
                        nc.scalar.activation(ost, ps_o, AF.Copy)
                        nc.scalar.dma_start(
                            out_d[:, dt_i, cs:cs + CH], ost)

            # ---- emission schedule.  Only qk(0,0) is needed before the
            # first score; everything else is queued as backfill atoms
            # pumped one-per-attention-block.  V(c) atoms go first in
            # chunk c's own queue (AV j-blocks >= 4c need them); next
            # chunk's qk follows; output projections fill the late,
            # ACT-bound chunks where the PE otherwise idles.
            chain_q(0, 0)()
            chain_k(0, 0)()

            backfill.extend([chain_v(0, si) for si in range(0, 4)])
            attn(0, 0)
            backfill.extend([chain_q(0, 1), chain_k(0, 1)])
            backfill.extend([chain_q(1, 0), chain_k(1, 0)])
            attn(0, 1)

            backfill.extend([chain_v(1, si) for si in range(4, 8)])
            backfill.extend([chain_q(1, 1), chain_k(1, 1)])
            backfill.extend([chain_q(2, 0), chain_k(2, 0)])
            attn(1, 0)
            backfill.extend([pd_group(0, i) for i in range(DT)])
            attn(1, 1)

            backfill.extend([chain_v(2, si) for si in range(8, 12)])
            backfill.extend([chain_q(2, 1), chain_k(2, 1)])
            backfill.extend([chain_q(3, 0), chain_k(3, 0)])
            attn(2, 0)
            backfill.extend([pd_group(1, i) for i in range(DT)])
            attn(2, 1)

            backfill.extend([chain_v(3, si) for si in range(12, 16)])
            backfill.extend([chain_q(3, 1), chain_k(3, 1)])
            backfill.extend([pd_group(2, i) for i in range(DT)])
            attn(3, 0)
            attn(3, 1)

            while backfill:
                pump()
            phase_d_last()

    nc.compile()
    return nc


def _tile128(a, inner_shape):
    """[N*128, ...] -> [128, N, ...] partition-major layout."""
    n = a.shape[0] // 128
    return np.ascontiguousarray(
        a.reshape((n, 128) + a.shape[1:]).swapaxes(0, 1)).reshape(
            (128, n) + inner_shape)


def _prep_core(x, W_Q, W_K, W_V, W_O, b_Q, b_K, b, g):
    hs = slice(g * HG, (g + 1) * HG)
    bfl = ml_dtypes.bfloat16

    xtp = np.ascontiguousarray(x[b].T)                       # [D, S]
    xt = _tile128(xtp, (S,)).astype(bfl)                     # [128, DT, S]
    # -> chunk-major [128, c, di, 512]
    xt = np.ascontiguousarray(
        xt.reshape(128, DT, NCH, CH).transpose(0, 2, 1, 3))

    def prep_w(w):                                           # [H,D,dh] slice
        wc = np.ascontiguousarray(
            w[hs].transpose(1, 0, 2).reshape(D, HK))         # [D, HK]
        return _tile128(wc, (HK,)).astype(bfl)               # [128, DT, HK]

    wq, wk, wv = prep_w(W_Q), prep_w(W_K), prep_w(W_V)
    woc = W_O[hs].reshape(HK, D)                             # [HK, D]
    wo = _tile128(woc, (D,)).astype(bfl)                     # [128, 2, D]

    bq = np.ascontiguousarray(
        b_Q[hs].reshape(HK).reshape(2, 128).T).astype(np.float32)
    bk = np.ascontiguousarray(
        b_K[hs].reshape(HK).reshape(2, 128).T).astype(np.float32)

    # relative tril mask for one 128-key diagonal band
    r = np.arange(128)[:, None, None]
    f = np.arange(KB)[None, None, :]
    mask = np.repeat((f >= r), 2, axis=1).astype(bfl)        # [128, 2, 128]

    return {"xt": xt, "wq": wq, "wk": wk, "wv": wv, "wo": wo,
            "bq": bq, "bk": bk, "mask": mask}


def kernel(x, W_Q, W_K, W_V, W_O, b_Q, b_K, b_V, b_O, **run_kwargs):
    x = np.asarray(x, dtype=np.float32)
    W_Q = np.asarray(W_Q, dtype=np.float32)
    W_K = np.asarray(W_K, dtype=np.float32)
    W_V = np.asarray(W_V, dtype=np.float32)
    W_O = np.asarray(W_O, dtype=np.float32)
    b_Q = np.asarray(b_Q, dtype=np.float32)
    b_K = np.asarray(b_K, dtype=np.float32)
    b_V = np.asarray(b_V, dtype=np.float32)
    b_O = np.asarray(b_O, dtype=np.float32)

    if "nc" not in _CACHE:
        _CACHE["nc"] = _build_nc()
    nc = _CACHE["nc"]

    in_maps = []
    for i in range(NCORES):
        b, g = i // HG, i % HG
        in_maps.append(_prep_core(x, W_Q, W_K, W_V, W_O, b_Q, b_K, b, g))

    res = run_bass_kernel_spmd(nc, in_maps, core_ids=list(range(NCORES)),
                               **run_kwargs)

    # exact fold of b_V through W_O (softmax rows sum to 1), plus b_O
    bias = (b_O.astype(np.float64)
            + b_V.reshape(H * DH).astype(np.float64)
            @ W_O.reshape(H * DH, D).astype(np.float64)).astype(np.float32)

    out = np.zeros((B, S, D), dtype=np.float32)
    for i in range(NCORES):
        b = i // HG
        # outT layout [128(p), DT, S] -> [S, D] with d = dt*128 + p
        o = res.results[i]["outT"].astype(np.float32)
        out[b] += o.transpose(2, 1, 0).reshape(S, D)
    out += bias[None, None, :]
    if run_kwargs:
        return out, res
    return out


# revision 14
# speedup vs baseline: 1.0218x; 1.0218x over previous
"""Multi-head causal attention (B=2, S=2048, D=1024, H=16, dh=64) on 8
Trainium2 NeuronCores.

Sharding: core i handles batch b = i//4 and head group g = i%4 (4 heads
each).  Per core everything is computed in a transposed layout:

  QT = Wq_g^T @ x_b^T          [256(hk), 2048(S)]   (bf16)
  KT = Wk_g^T @ x_b^T          [256(hk), 2048(S)]   (bf16)
  V  = x_b @ Wv_g              [2048(S), 4, 65]     (bf16; col 64 = ones)
  per chunk c (512 queries), head-pair hp, key block j (128 keys):
     scT[par] = KT_h[:,j]^T(lhsT) x QT_h[:,c]   -> PSUM [128, 2, 512]
                (the two heads of a pair use PE rows 0-63 / 64-127 and
                 run concurrently)
     expT     = exp(scT/8) (* causal mask when j >= 4c)        (bf16)
     zT_h    += V_aug[j]^T(lhsT) x expT[par]    -> PSUM [65, 512]
                (row 64 accumulates the softmax denominator s)
     ztn      = zT[0:64] * broadcast(1/s)       [256(hk), 2048] (bf16)
  outT(c) = Wo_g^T(lhsT) x ztn(c)               [1024(d), 512]  (bf16)

v2 scheduling changes vs v1:
  - The scalar engine is the pacer of the attention stretch (exp runs at
    (N+352)/1.2 ns and is ACT-only), so ALL other ACT work was moved off
    it: Q/K PSUM drains (+bias) now run on DVE via tensor_scalar_add.
  - Input DMA is split across both HWDGE rings (sync + scalar) with the
    critical prefix first (wq/wk m0 halves, x chunk 0 per-di), so the
    first exp fires ~8us earlier.
  - phase_d(c) is emitted shift-1 (after attn(c+1,1)) so output
    projections backfill PE idle slots inside the ACT-paced stretch
    instead of forming a serial tail.  Chunks 0-2 collect into a
    per-chunk SBUF buffer shipped by one DMA; the last chunk drains
    per-dt with copies alternating ACT/DVE and DMAs alternating
    sync/scalar rings to shorten the drain.

Host: shards/transposes inputs, sums the 4 head-group partial outputs
per batch, adds b_O and the exact b_V fold (softmax rows sum to 1):
  out += b_O + sum_h b_V[h] @ W_O[h].
"""
import numpy as np
import ml_dtypes

import concourse.bacc as bacc
import concourse.mybir as mybir
import concourse.tile as tile
from concourse.bass_utils import run_bass_kernel_spmd

f32 = mybir.dt.float32
bf16 = mybir.dt.bfloat16
AF = mybir.ActivationFunctionType

B, S, D, H, DH = 2, 2048, 1024, 16, 64
NCORES = 8
HG = 4                # heads per core
HK = HG * DH          # 256
CH = 512              # query chunk
NCH = S // CH         # 4
KB = 128              # key block
DT = D // 128         # 8

_CACHE = {}


def _build_nc():
    nc = bacc.Bacc(None, target_bir_lowering=False, debug=False,
                   num_devices=NCORES)

    # x^T tiled chunk-major: [128, c, di, 512]
    xt_d = nc.dram_tensor("xt", [128, NCH, DT, CH], bf16,
                          kind="ExternalInput")
    wq_d = nc.dram_tensor("wq", [128, DT, HK], bf16, kind="ExternalInput")
    wk_d = nc.dram_tensor("wk", [128, DT, HK], bf16, kind="ExternalInput")
    wv_d = nc.dram_tensor("wv", [128, DT, HK], bf16, kind="ExternalInput")
    wo_d = nc.dram_tensor("wo", [128, 2, D], bf16, kind="ExternalInput")
    bq_d = nc.dram_tensor("bq", [128, 2], f32, kind="ExternalInput")
    bk_d = nc.dram_tensor("bk", [128, 2], f32, kind="ExternalInput")
    # relative causal mask for a 128-key diagonal band: [128, 2(par), 128]
    mask_d = nc.dram_tensor("mask", [128, 2, KB], bf16, kind="ExternalInput")
    out_d = nc.dram_tensor("outT", [128, DT, S], bf16, kind="ExternalOutput")

    with tile.TileContext(nc) as tc:
        with (
            tc.tile_pool(name="const", bufs=1) as cp,
            tc.tile_pool(name="big", bufs=1) as bp,
            tc.tile_pool(name="work", bufs=3) as wp,
            tc.tile_pool(name="psum", bufs=2, space="PSUM") as pp,
        ):
            wq = cp.tile([128, DT, HK], bf16)
            wk = cp.tile([128, DT, HK], bf16)
            wv = cp.tile([128, DT, HK], bf16)
            wo = cp.tile([128, 2, D], bf16)
            bq = cp.tile([128, 2], f32)
            bk = cp.tile([128, 2], f32)
            mask = cp.tile([128, 2, KB], bf16)
            xt = bp.tile([128, NCH, DT, CH], bf16)

            # ---- input DMA, split across both HWDGE rings plus the
            # SWDGE (gpsimd) ring.  HBM bandwidth is the binding
            # resource at startup, so the critical prefix for the first
            # projection chain (Wq/Wk m0 halves + ALL of x chunk 0) goes
            # first on BOTH rings; everything else strictly after.  Wv
            # rides HWDGE too (SWDGE is ~100GB/s and would deliver it
            # too late for the first AV matmul).
            nc.sync.dma_start(wq[:, :, 0:128], wq_d[:, :, 0:128])
            nc.scalar.dma_start(wk[:, :, 0:128], wk_d[:, :, 0:128])
            for p in range(2):
                nc.sync.dma_start(xt[:, 0, 2 * p:2 * p + 2, :],
                                  xt_d[:, 0, 2 * p:2 * p + 2, :])
                nc.scalar.dma_start(xt[:, 0, 4 + 2 * p:6 + 2 * p, :],
                                    xt_d[:, 0, 4 + 2 * p:6 + 2 * p, :])
            nc.sync.dma_start(wq[:, :, 128:256], wq_d[:, :, 128:256])
            nc.scalar.dma_start(wv, wv_d[:])
            nc.sync.dma_start(xt[:, 1], xt_d[:, 1])
            nc.scalar.dma_start(wk[:, :, 128:256], wk_d[:, :, 128:256])
            nc.sync.dma_start(xt[:, 2], xt_d[:, 2])
            nc.scalar.dma_start(xt[:, 3], xt_d[:, 3])
            nc.gpsimd.dma_start(bq, bq_d[:])
            nc.gpsimd.dma_start(bk, bk_d[:])
            nc.gpsimd.dma_start(mask, mask_d[:])
            nc.gpsimd.dma_start(wo, wo_d[:])

            qt = bp.tile([128, 2, S], bf16)
            kt = bp.tile([128, 2, S], bf16)
            # V padded to 128 columns (col 64 = ones for the softmax
            # denominator; 65.. zero) so the zT matmul's stationary is
            # 128-wide -> fast weight load
            v = bp.tile([128, S // KB, HG, 128], bf16)
            ztn = bp.tile([128, 2, S], bf16)

            # v padding (ones + zeros) on gpsimd: it is idle at startup
            # and this must NOT delay the DVE memsets that gate the
            # HAM warm-up matmuls.
            nc.gpsimd.memset(v[:, :, :, DH:DH + 1], 1.0)
            nc.gpsimd.memset(v[:, :, :, DH + 1:], 0.0)

            # preload the exp ACT table set at t=0 (one-time ~2.7us,
            # overlapped with input DMA)
            warm = wp.tile([1, 1], f32, tag="warm", bufs=1)
            nc.vector.memset(warm, 0.0)
            nc.scalar.activation(warm, warm, AF.Exp)

            # HAM warm-up: dummy matmuls from PE program start until the
            # first projections' DMA lands keep the PE clock gate open,
            # so real work runs at 2.4GHz from the first instruction.
            wrm = wp.tile([128, 128], bf16, tag="wrm", bufs=1)
            nc.vector.memset(wrm, 0.0)
            ps_w = pp.tile([128, KB], f32, tag="proj", name="ps_warm")
            for r in range(40):
                nc.tensor.matmul(ps_w, wrm, wrm, start=(r == 0),
                                 stop=(r == 39))

            # ---- backfill machinery.  The Tile scheduler builds static
            # in-order per-engine programs (priority = emission order
            # among data-ready ops), so projection / output-projection
            # work must be emitted in SMALL atoms interleaved between
            # attention blocks — otherwise it serializes at chunk
            # boundaries and starves the exp-paced scalar engine.
            backfill = []

            def pump():
                if backfill:
                    backfill.pop(0)()

            def chain_q(c, m):
                def th():
                    cs = c * CH
                    ps_q = pp.tile([128, CH], f32, tag="proj",
                                   name=f"ps_q_{c}_{m}")
                    for di in range(DT):
                        nc.tensor.matmul(
                            ps_q, wq[:, di, m * 128:(m + 1) * 128],
                            xt[:, c, di, :],
                            start=(di == 0), stop=(di == DT - 1))
                    nc.vector.tensor_scalar_add(
                        out=qt[:, m, cs:cs + CH], in0=ps_q,
                        scalar1=bq[:, m:m + 1])
                return th

            def chain_k(c, m):
                def th():
                    cs = c * CH
                    ps_k = pp.tile([128, CH], f32, tag="proj",
                                   name=f"ps_k_{c}_{m}")
                    for di in range(DT):
                        nc.tensor.matmul(
                            ps_k, wk[:, di, m * 128:(m + 1) * 128],
                            xt[:, c, di, :],
                            start=(di == 0), stop=(di == DT - 1))
                    nc.vector.tensor_scalar_add(
                        out=kt[:, m, cs:cs + CH], in0=ps_k,
                        scalar1=bk[:, m:m + 1])
                return th

            def chain_v(c, si):
                def th():
                    sl = (si - 4 * c) * KB
                    ps_v = pp.tile([128, HG, DH], f32, tag="proj",
                                   name=f"ps_v_{si}")
                    for di in range(DT):
                        nc.tensor.matmul(
                            ps_v, xt[:, c, di, sl:sl + KB], wv[:, di, :],
                            start=(di == 0), stop=(di == DT - 1))
                    nc.vector.tensor_copy(v[:, si, :, 0:DH], ps_v)
                return th

            ostc_tiles = {}

            def pd_group(c, dt_i):
                """One output-projection group (chunks 0..NCH-2): two
                accumulating matmuls + DVE drain into the per-chunk
                staging buffer; the last group ships the whole chunk
                with a single sync-ring DMA."""
                def th():
                    cs = c * CH
                    if c not in ostc_tiles:
                        ostc_tiles[c] = wp.tile(
                            [128, DT, CH], bf16, tag="ostc", bufs=2,
                            name=f"ostc_{c}")
                    ostc = ostc_tiles[c]
                    ps_o = pp.tile([128, CH], f32, tag="proj",
                                   name=f"ps_o_{c}_{dt_i}")
                    for m in range(2):
                        nc.tensor.matmul(
                            ps_o, wo[:, m, dt_i * 128:(dt_i + 1) * 128],
                            ztn[:, m, cs:cs + CH],
                            start=(m == 0), stop=(m == 1))
                    nc.vector.tensor_copy(ostc[:, dt_i, :], ps_o)
                    if dt_i == DT - 1:
                        nc.sync.dma_start(out_d[:, :, cs:cs + CH], ostc)
                return th

            def attn(c, hp):
                """Attention for (query chunk c, head pair hp)."""
                cs = c * CH
                m = hp
                nblk = 4 * c + 4
                zt0 = pp.tile([128, CH], f32, tag="zt0", bufs=1,
                              name=f"zt0_{c}_{hp}")
                zt1 = pp.tile([128, CH], f32, tag="zt1", bufs=1,
                              name=f"zt1_{c}_{hp}")
                zts = (zt0, zt1)
                for j in range(nblk):
                    pump()
                    # diagonal blocks (t>=0): queries below 128t are fully
                    # masked -> compute only [128t, CH)
                    t = j - 4 * c
                    ql = 128 * t if t > 0 else 0
                    sc = pp.tile([128, 2, CH], f32, tag="sc",
                                 name=f"sc_{c}_{hp}_{j}")
                    for par in range(2):
                        o = par * 64
                        nc.tensor.matmul(
                            sc[:, par, ql:],
                            kt[o:o + 64, m, j * KB:(j + 1) * KB],
                            qt[o:o + 64, m, cs + ql:cs + CH],
                            start=True, stop=True)
                    ex = wp.tile([128, 2, CH], bf16, tag="ex", bufs=6)
                    nc.scalar.activation(ex[:, :, ql:], sc[:, :, ql:],
                                         AF.Exp, scale=0.125)
                    if t >= 0:
                        qm = ql + 128
                        nc.vector.tensor_mul(ex[:, :, ql:qm],
                                             ex[:, :, ql:qm], mask)
                    for par in range(2):
                        h = 2 * hp + par
                        nc.tensor.matmul(
                            zts[par][:, ql:], v[:, j, h, :],
                            ex[:, par, ql:],
                            start=(j == 0), stop=(j == nblk - 1))
                # normalize: ztn[h] = zt[0:64] / zt[64].  First bounce the
                # zT+denominator block to SBUF so the PSUM accumulator
                # frees immediately for the next head pair; the divide
                # chain then runs from SBUF.  The very last pair has no
                # successor wanting its PSUM slot, so skip the bounce.
                last = (c == NCH - 1 and hp == 1)
                for par in range(2):
                    h = 2 * hp + par
                    o = par * 64
                    if last:
                        zsrc = zts[par]
                    else:
                        zs = wp.tile([DH + 1, CH], f32, tag="zs", bufs=3,
                                     name=f"zs_{c}_{h}")
                        nc.vector.tensor_copy(zs, zts[par][0:DH + 1, :])
                        zsrc = zs
                    srow = wp.tile([1, CH], f32, tag="srow", bufs=3,
                                   name=f"srow_{c}_{h}")
                    nc.vector.tensor_copy(srow, zsrc[DH:DH + 1, :])
                    rec = wp.tile([1, CH], f32, tag="rec", bufs=3,
                                  name=f"rec_{c}_{h}")
                    nc.vector.reciprocal_approx_fast(rec, srow)
                    bc = wp.tile([64, CH], f32, tag="bc", bufs=3,
                                 name=f"bc_{c}_{h}")
                    nc.gpsimd.partition_broadcast(bc, rec)
                    nc.vector.tensor_mul(ztn[o:o + 64, m, cs:cs + CH],
                                         zsrc[0:DH, :], bc)

            def phase_d_last():
                """Output projection for the last chunk: per-dt drain
                with copies alternating DVE/ACT (ACT is idle in the
                tail) and DMAs alternating sync/scalar rings."""
                c = NCH - 1
                cs = c * CH
                for dt_i in range(DT):
                    ps_o = pp.tile([128, CH], f32, tag="proj",
                                   name=f"ps_o_{c}_{dt_i}")
                    for m in range(2):
                        nc.tensor.matmul(
                            ps_o, wo[:, m, dt_i * 128:(dt_i + 1) * 128],
                            ztn[:, m, cs:cs + CH],
                            start=(m == 0), stop=(m == 1))
                    ost = wp.tile([128, CH], bf16, tag="ost", bufs=4,
                                  name=f"ost_{c}_{dt_i}")
                    if dt_i % 2 == 0:
                        nc.vector.tensor_copy(ost, ps_o)
                        nc.sync.dma_start(
                            out_d[:, dt_i, cs:cs + CH], ost)
                    else:
                        nc.scalar.activation(ost, ps_o, AF.Copy)
                        nc.scalar.dma_start(
                            out_d[:, dt_i, cs:cs + CH], ost)

            # ---- emission schedule.  Only qk(0,0) is needed before the
            # first score; everything else is queued as backfill atoms
            # pumped one-per-attention-block.  V(c) atoms go first in
            # chunk c's own queue (AV j-blocks >= 4c need them); next
            # chunk's qk follows; output projections fill the late,
            # ACT-bound chunks where the PE otherwise idles.
            chain_q(0, 0)()
            chain_k(0, 0)()

            backfill.extend([chain_v(0, si) for si in range(0, 4)])
            attn(0, 0)
            backfill.extend([chain_q(0, 1), chain_k(0, 1)])
            backfill.extend([chain_q(1, 0), chain_k(1, 0)])
            attn(0, 1)

            backfill.extend([chain_v(1, si) for si in range(4, 8)])
            backfill.extend([chain_q(1, 1), chain_k(1, 1)])
            backfill.extend([chain_q(2, 0), chain_k(2, 0)])
            attn(1, 0)
            backfill.extend([pd_group(0, i) for i in range(DT)])
            attn(1, 1)

            backfill.extend([chain_v(2, si) for si in range(8, 12)])
            backfill.extend([chain_q(2, 1), chain_k(2, 1)])
            backfill.extend([chain_q(3, 0), chain_k(3, 0)])
            attn(2, 0)
            backfill.extend([pd_group(1, i) for i in range(DT)])
            attn(2, 1)

            backfill.extend([chain_v(3, si) for si in range(12, 16)])
            backfill.extend([chain_q(3, 1), chain_k(3, 1)])
            attn(3, 0)
            backfill.extend([pd_group(2, i) for i in range(DT)])
            attn(3, 1)

            while backfill:
                pump()
            phase_d_last()

    nc.compile()
    return nc


def _tile128(a, inner_shape):
    """[N*128, ...] -> [128, N, ...] partition-major layout."""
    n = a.shape[0] // 128
    return np.ascontiguousarray(
        a.reshape((n, 128) + a.shape[1:]).swapaxes(0, 1)).reshape(
            (128, n) + inner_shape)


def _prep_core(x, W_Q, W_K, W_V, W_O, b_Q, b_K, b, g):
    hs = slice(g * HG, (g + 1) * HG)
    bfl = ml_dtypes.bfloat16

    xtp = np.ascontiguousarray(x[b].T)                       # [D, S]
    xt = _tile128(xtp, (S,)).astype(bfl)                     # [128, DT, S]
    # -> chunk-major [128, c, di, 512]
    xt = np.ascontiguousarray(
        xt.reshape(128, DT, NCH, CH).transpose(0, 2, 1, 3))

    def prep_w(w):                                           # [H,D,dh] slice
        wc = np.ascontiguousarray(
            w[hs].transpose(1, 0, 2).reshape(D, HK))         # [D, HK]
        return _tile128(wc, (HK,)).astype(bfl)               # [128, DT, HK]

    wq, wk, wv = prep_w(W_Q), prep_w(W_K), prep_w(W_V)
    woc = W_O[hs].reshape(HK, D)                             # [HK, D]
    wo = _tile128(woc, (D,)).astype(bfl)                     # [128, 2, D]

    bq = np.ascontiguousarray(
        b_Q[hs].reshape(HK).reshape(2, 128).T).astype(np.float32)
    bk = np.ascontiguousarray(
        b_K[hs].reshape(HK).reshape(2, 128).T).astype(np.float32)

    # relative tril mask for one 128-key diagonal band
    r = np.arange(128)[:, None, None]
    f = np.arange(KB)[None, None, :]
    mask = np.repeat((f >= r), 2, axis=1).astype(bfl)        # [128, 2, 128]

    return {"xt": xt, "wq": wq, "wk": wk, "wv": wv, "wo": wo,
            "bq": bq, "bk": bk, "mask": mask}


def kernel(x, W_Q, W_K, W_V, W_O, b_Q, b_K, b_V, b_O, **run_kwargs):
    x = np.asarray(x, dtype=np.float32)
    W_Q = np.asarray(W_Q, dtype=np.float32)
    W_K = np.asarray(W_K, dtype=np.float32)
    W_V = np.asarray(W_V, dtype=np.float32)
    W_O = np.asarray(W_O, dtype=np.float32)
    b_Q = np.asarray(b_Q, dtype=np.float32)
    b_K = np.asarray(b_K, dtype=np.float32)
    b_V = np.asarray(b_V, dtype=np.float32)
    b_O = np.asarray(b_O, dtype=np.float32)

    if "nc" not in _CACHE:
        _CACHE["nc"] = _build_nc()
    nc = _CACHE["nc"]

    in_maps = []
    for i in range(NCORES):
        b, g = i // HG, i % HG
        in_maps.append(_prep_core(x, W_Q, W_K, W_V, W_O, b_Q, b_K, b, g))

    res = run_bass_kernel_spmd(nc, in_maps, core_ids=list(range(NCORES)),
                               **run_kwargs)

    # exact fold of b_V through W_O (softmax rows sum to 1), plus b_O
    bias = (b_O.astype(np.float64)
            + b_V.reshape(H * DH).astype(np.float64)
            @ W_O.reshape(H * DH, D).astype(np.float64)).astype(np.float32)

    out = np.zeros((B, S, D), dtype=np.float32)
    for i in range(NCORES):
        b = i // HG
        # outT layout [128(p), DT, S] -> [S, D] with d = dt*128 + p
        o = res.results[i]["outT"].astype(np.float32)
        out[b] += o.transpose(2, 1, 0).reshape(S, D)
    out += bias[None, None, :]
    if run_kwargs:
        return out, res
    return out


# revision 18
# speedup vs baseline: 1.0422x; 1.0200x over previous
"""Multi-head causal attention (B=2, S=2048, D=1024, H=16, dh=64) on 8
Trainium2 NeuronCores.

Sharding: core i handles batch b = i//4 and head group g = i%4 (4 heads
each).  Per core everything is computed in a transposed layout:

  QT = Wq_g^T @ x_b^T          [256(hk), 2048(S)]   (bf16)
  KT = Wk_g^T @ x_b^T          [256(hk), 2048(S)]   (bf16)
  V  = x_b @ Wv_g              [2048(S), 4, 65]     (bf16; col 64 = ones)
  per chunk c (512 queries), head-pair hp, key block j (128 keys):
     scT[par] = KT_h[:,j]^T(lhsT) x QT_h[:,c]   -> PSUM [128, 2, 512]
                (the two heads of a pair use PE rows 0-63 / 64-127 and
                 run concurrently)
     expT     = exp(scT/8) (* causal mask when j >= 4c)        (bf16)
     zT_h    += V_aug[j]^T(lhsT) x expT[par]    -> PSUM [65, 512]
                (row 64 accumulates the softmax denominator s)
     ztn      = zT[0:64] * broadcast(1/s)       [256(hk), 2048] (bf16)
  outT(c) = Wo_g^T(lhsT) x ztn(c)               [1024(d), 512]  (bf16)

v2 scheduling changes vs v1:
  - The scalar engine is the pacer of the attention stretch (exp runs at
    (N+352)/1.2 ns and is ACT-only), so ALL other ACT work was moved off
    it: Q/K PSUM drains (+bias) now run on DVE via tensor_scalar_add.
  - Input DMA is split across both HWDGE rings (sync + scalar) with the
    critical prefix first (wq/wk m0 halves, x chunk 0 per-di), so the
    first exp fires ~8us earlier.
  - phase_d(c) is emitted shift-1 (after attn(c+1,1)) so output
    projections backfill PE idle slots inside the ACT-paced stretch
    instead of forming a serial tail.  Chunks 0-2 collect into a
    per-chunk SBUF buffer shipped by one DMA; the last chunk drains
    per-dt with copies alternating ACT/DVE and DMAs alternating
    sync/scalar rings to shorten the drain.

Host: shards/transposes inputs, sums the 4 head-group partial outputs
per batch, adds b_O and the exact b_V fold (softmax rows sum to 1):
  out += b_O + sum_h b_V[h] @ W_O[h].
"""
import numpy as np
import ml_dtypes

import concourse.bacc as bacc
import concourse.mybir as mybir
import concourse.tile as tile
from concourse.bass_utils import run_bass_kernel_spmd

f32 = mybir.dt.float32
bf16 = mybir.dt.bfloat16
AF = mybir.ActivationFunctionType

B, S, D, H, DH = 2, 2048, 1024, 16, 64
NCORES = 8
HG = 4                # heads per core
HK = HG * DH          # 256
CH = 512              # query chunk
NCH = S // CH         # 4
KB = 128              # key block
DT = D // 128         # 8

_CACHE = {}


def _build_nc():
    nc = bacc.Bacc(None, target_bir_lowering=False, debug=False,
                   num_devices=NCORES)

    # x^T tiled chunk-major: [128, c, di, 512]
    xt_d = nc.dram_tensor("xt", [128, NCH, DT, CH], bf16,
                          kind="ExternalInput")
    wq_d = nc.dram_tensor("wq", [128, DT, HK], bf16, kind="ExternalInput")
    wk_d = nc.dram_tensor("wk", [128, DT, HK], bf16, kind="ExternalInput")
    wv_d = nc.dram_tensor("wv", [128, DT, HK], bf16, kind="ExternalInput")
    wo_d = nc.dram_tensor("wo", [128, 2, D], bf16, kind="ExternalInput")
    bq_d = nc.dram_tensor("bq", [128, 2], f32, kind="ExternalInput")
    bk_d = nc.dram_tensor("bk", [128, 2], f32, kind="ExternalInput")
    # relative causal mask for a 128-key diagonal band: [128, 2(par), 128]
    mask_d = nc.dram_tensor("mask", [128, 2, KB], bf16, kind="ExternalInput")
    out_d = nc.dram_tensor("outT", [128, DT, S], bf16, kind="ExternalOutput")

    with tile.TileContext(nc) as tc:
        with (
            tc.tile_pool(name="const", bufs=1) as cp,
            tc.tile_pool(name="big", bufs=1) as bp,
            tc.tile_pool(name="work", bufs=3) as wp,
            tc.tile_pool(name="psum", bufs=2, space="PSUM") as pp,
        ):
            wq = cp.tile([128, DT, HK], bf16)
            wk = cp.tile([128, DT, HK], bf16)
            wv = cp.tile([128, DT, HK], bf16)
            wo = cp.tile([128, 2, D], bf16)
            bq = cp.tile([128, 2], f32)
            bk = cp.tile([128, 2], f32)
            mask = cp.tile([128, 2, KB], bf16)
            xt = bp.tile([128, NCH, DT, CH], bf16)

            # ---- input DMA, split across both HWDGE rings plus the
            # SWDGE (gpsimd) ring.  HBM bandwidth is the binding
            # resource at startup, so the critical prefix for the first
            # projection chain (Wq/Wk m0 halves + ALL of x chunk 0) goes
            # first on BOTH rings; everything else strictly after.  Wv
            # rides HWDGE too (SWDGE is ~100GB/s and would deliver it
            # too late for the first AV matmul).
            nc.sync.dma_start(wq[:, :, 0:128], wq_d[:, :, 0:128])
            nc.scalar.dma_start(wk[:, :, 0:128], wk_d[:, :, 0:128])
            for p in range(2):
                nc.sync.dma_start(xt[:, 0, 2 * p:2 * p + 2, :],
                                  xt_d[:, 0, 2 * p:2 * p + 2, :])
                nc.scalar.dma_start(xt[:, 0, 4 + 2 * p:6 + 2 * p, :],
                                    xt_d[:, 0, 4 + 2 * p:6 + 2 * p, :])
            nc.sync.dma_start(wq[:, :, 128:256], wq_d[:, :, 128:256])
            nc.scalar.dma_start(wv, wv_d[:])
            nc.sync.dma_start(xt[:, 1], xt_d[:, 1])
            nc.scalar.dma_start(wk[:, :, 128:256], wk_d[:, :, 128:256])
            nc.sync.dma_start(xt[:, 2], xt_d[:, 2])
            nc.scalar.dma_start(xt[:, 3], xt_d[:, 3])
            nc.gpsimd.dma_start(bq, bq_d[:])
            nc.gpsimd.dma_start(bk, bk_d[:])
            nc.gpsimd.dma_start(mask, mask_d[:])
            nc.gpsimd.dma_start(wo, wo_d[:])

            qt = bp.tile([128, 2, S], bf16)
            kt = bp.tile([128, 2, S], bf16)
            # V padded to 128 columns, cols 64..127 ALL ones: the zT
            # matmul then replicates the softmax denominator s into PSUM
            # rows 64..127 for free, so the normalize can run a
            # partition-parallel reciprocal with no gpsimd broadcast.
            v = bp.tile([128, S // KB, HG, 128], bf16)
            ztn = bp.tile([128, 2, S], bf16)

            # (on gpsimd: idle at startup, must not delay the DVE
            # memsets that gate the HAM warm-up matmuls)
            nc.gpsimd.memset(v[:, :, :, DH:], 1.0)

            # preload the exp ACT table set at t=0 (one-time ~2.7us,
            # overlapped with input DMA)
            warm = wp.tile([1, 1], f32, tag="warm", bufs=1)
            nc.vector.memset(warm, 0.0)
            nc.scalar.activation(warm, warm, AF.Exp)

            # HAM warm-up: dummy matmuls from PE program start until the
            # first projections' DMA lands keep the PE clock gate open,
            # so real work runs at 2.4GHz from the first instruction.
            wrm = wp.tile([128, 128], bf16, tag="wrm", bufs=1)
            nc.vector.memset(wrm, 0.0)
            ps_w = pp.tile([128, KB], f32, tag="proj", name="ps_warm")
            for r in range(28):
                nc.tensor.matmul(ps_w, wrm, wrm, start=(r == 0),
                                 stop=(r == 27))

            # ---- backfill machinery.  The Tile scheduler builds static
            # in-order per-engine programs (priority = emission order
            # among data-ready ops), so projection / output-projection
            # work must be emitted in SMALL atoms interleaved between
            # attention blocks — otherwise it serializes at chunk
            # boundaries and starves the exp-paced scalar engine.
            backfill = []

            def pump():
                if backfill:
                    backfill.pop(0)()

            def chain_q(c, m):
                def th():
                    cs = c * CH
                    ps_q = pp.tile([128, CH], f32, tag="proj",
                                   name=f"ps_q_{c}_{m}")
                    for di in range(DT):
                        nc.tensor.matmul(
                            ps_q, wq[:, di, m * 128:(m + 1) * 128],
                            xt[:, c, di, :],
                            start=(di == 0), stop=(di == DT - 1))
                    nc.vector.tensor_scalar_add(
                        out=qt[:, m, cs:cs + CH], in0=ps_q,
                        scalar1=bq[:, m:m + 1])
                return th

            def chain_k(c, m):
                def th():
                    cs = c * CH
                    ps_k = pp.tile([128, CH], f32, tag="proj",
                                   name=f"ps_k_{c}_{m}")
                    for di in range(DT):
                        nc.tensor.matmul(
                            ps_k, wk[:, di, m * 128:(m + 1) * 128],
                            xt[:, c, di, :],
                            start=(di == 0), stop=(di == DT - 1))
                    nc.vector.tensor_scalar_add(
                        out=kt[:, m, cs:cs + CH], in0=ps_k,
                        scalar1=bk[:, m:m + 1])
                return th

            def chain_v(c, si):
                def th():
                    sl = (si - 4 * c) * KB
                    ps_v = pp.tile([128, HG, DH], f32, tag="proj",
                                   name=f"ps_v_{si}")
                    for di in range(DT):
                        nc.tensor.matmul(
                            ps_v, xt[:, c, di, sl:sl + KB], wv[:, di, :],
                            start=(di == 0), stop=(di == DT - 1))
                    nc.vector.tensor_copy(v[:, si, :, 0:DH], ps_v)
                return th

            ostc_tiles = {}

            def pd_group(c, dt_i):
                """One output-projection group (chunks 0..NCH-2): two
                accumulating matmuls + DVE drain into the per-chunk
                staging buffer; the last group ships the whole chunk
                with a single sync-ring DMA."""
                def th():
                    cs = c * CH
                    if c not in ostc_tiles:
                        ostc_tiles[c] = wp.tile(
                            [128, DT, CH], bf16, tag="ostc", bufs=2,
                            name=f"ostc_{c}")
                    ostc = ostc_tiles[c]
                    ps_o = pp.tile([128, CH], f32, tag="proj",
                                   name=f"ps_o_{c}_{dt_i}")
                    for m in range(2):
                        nc.tensor.matmul(
                            ps_o, wo[:, m, dt_i * 128:(dt_i + 1) * 128],
                            ztn[:, m, cs:cs + CH],
                            start=(m == 0), stop=(m == 1))
                    nc.vector.tensor_copy(ostc[:, dt_i, :], ps_o)
                    if dt_i == DT - 1:
                        nc.sync.dma_start(out_d[:, :, cs:cs + CH], ostc)
                return th

            def attn(c, hp):
                """Attention for (query chunk c, head pair hp)."""
                cs = c * CH
                m = hp
                nblk = 4 * c + 4
                zt0 = pp.tile([128, CH], f32, tag="zt0", bufs=1,
                              name=f"zt0_{c}_{hp}")
                zt1 = pp.tile([128, CH], f32, tag="zt1", bufs=1,
                              name=f"zt1_{c}_{hp}")
                zts = (zt0, zt1)
                for j in range(nblk):
                    pump()
                    # diagonal blocks (t>=0): queries below 128t are fully
                    # masked -> compute only [128t, CH)
                    t = j - 4 * c
                    ql = 128 * t if t > 0 else 0
                    sc = pp.tile([128, 2, CH], f32, tag="sc",
                                 name=f"sc_{c}_{hp}_{j}")
                    for par in range(2):
                        o = par * 64
                        nc.tensor.matmul(
                            sc[:, par, ql:],
                            kt[o:o + 64, m, j * KB:(j + 1) * KB],
                            qt[o:o + 64, m, cs + ql:cs + CH],
                            start=True, stop=True)
                    ex = wp.tile([128, 2, CH], bf16, tag="ex", bufs=6)
                    nc.scalar.activation(ex[:, :, ql:], sc[:, :, ql:],
                                         AF.Exp, scale=0.125)
                    if t >= 0:
                        qm = ql + 128
                        nc.vector.tensor_mul(ex[:, :, ql:qm],
                                             ex[:, :, ql:qm], mask)
                    for par in range(2):
                        h = 2 * hp + par
                        nc.tensor.matmul(
                            zts[par][:, ql:], v[:, j, h, :],
                            ex[:, par, ql:],
                            start=(j == 0), stop=(j == nblk - 1))
                # normalize: ztn[h] = zt[0:64] / s, where s sits
                # REPLICATED in zt rows 64..127 (ones-block in V).  Per
                # par: aligned PSUM->SBUF copy of the s block, aligned
                # 64-lane reciprocal, then one multiply reading z
                # straight from PSUM (mixed psum/sbuf partition bases
                # are legal on the DVE).
                for par in range(2):
                    h = 2 * hp + par
                    o = par * 64
                    srep = wp.tile([64, CH], f32, tag="srep", bufs=2,
                                   name=f"srep_{c}_{h}")
                    nc.vector.tensor_copy(srep, zts[par][64:128, :])
                    srec = wp.tile([64, CH], f32, tag="srec", bufs=2,
                                   name=f"srec_{c}_{h}")
                    nc.vector.reciprocal_approx_fast(srec, srep)
                    nc.vector.tensor_mul(ztn[o:o + 64, m, cs:cs + CH],
                                         zts[par][0:DH, :], srec)

            def phase_d_last():
                """Output projection for the last chunk: per-dt drain
                with copies alternating DVE/ACT (ACT is idle in the
                tail) and DMAs alternating sync/scalar rings."""
                c = NCH - 1
                cs = c * CH
                for dt_i in range(DT):
                    ps_o = pp.tile([128, CH], f32, tag="proj",
                                   name=f"ps_o_{c}_{dt_i}")
                    for m in range(2):
                        nc.tensor.matmul(
                            ps_o, wo[:, m, dt_i * 128:(dt_i + 1) * 128],
                            ztn[:, m, cs:cs + CH],
                            start=(m == 0), stop=(m == 1))
                    ost = wp.tile([128, CH], bf16, tag="ost", bufs=4,
                                  name=f"ost_{c}_{dt_i}")
                    if dt_i % 2 == 0:
                        nc.vector.tensor_copy(ost, ps_o)
                        nc.sync.dma_start(
                            out_d[:, dt_i, cs:cs + CH], ost)
                    else:
                        nc.scalar.activation(ost, ps_o, AF.Copy)
                        nc.scalar.dma_start(
                            out_d[:, dt_i, cs:cs + CH], ost)

            # ---- emission schedule.  Only qk(0,0) is needed before the
            # first score; everything else is queued as backfill atoms
            # pumped one-per-attention-block.  V(c) atoms go first in
            # chunk c's own queue (AV j-blocks >= 4c need them); next
            # chunk's qk follows; output projections fill the late,
            # ACT-bound chunks where the PE otherwise idles.
            chain_q(0, 0)()
            chain_k(0, 0)()

            backfill.extend([chain_v(0, si) for si in range(0, 4)])
            attn(0, 0)
            backfill.extend([chain_q(0, 1), chain_k(0, 1)])
            backfill.extend([chain_q(1, 0), chain_k(1, 0)])
            attn(0, 1)

            backfill.extend([chain_v(1, si) for si in range(4, 8)])
            backfill.extend([chain_q(1, 1), chain_k(1, 1)])
            backfill.extend([chain_q(2, 0), chain_k(2, 0)])
            attn(1, 0)
            backfill.extend([pd_group(0, i) for i in range(DT)])
            attn(1, 1)

            backfill.extend([chain_v(2, si) for si in range(8, 12)])
            backfill.extend([chain_q(2, 1), chain_k(2, 1)])
            backfill.extend([chain_q(3, 0), chain_k(3, 0)])
            attn(2, 0)
            backfill.extend([pd_group(1, i) for i in range(DT)])
            attn(2, 1)

            backfill.extend([chain_v(3, si) for si in range(12, 16)])
            backfill.extend([chain_q(3, 1), chain_k(3, 1)])
            attn(3, 0)
            backfill.extend([pd_group(2, i) for i in range(DT)])
            attn(3, 1)

            while backfill:
                pump()
            phase_d_last()

    nc.compile()
    return nc


def _tile128(a, inner_shape):
    """[N*128, ...] -> [128, N, ...] partition-major layout."""
    n = a.shape[0] // 128
    return np.ascontiguousarray(
        a.reshape((n, 128) + a.shape[1:]).swapaxes(0, 1)).reshape(
            (128, n) + inner_shape)


def _prep_core(x, W_Q, W_K, W_V, W_O, b_Q, b_K, b, g):
    hs = slice(g * HG, (g + 1) * HG)
    bfl = ml_dtypes.bfloat16

    xtp = np.ascontiguousarray(x[b].T)                       # [D, S]
    xt = _tile128(xtp, (S,)).astype(bfl)                     # [128, DT, S]
    # -> chunk-major [128, c, di, 512]
    xt = np.ascontiguousarray(
        xt.reshape(128, DT, NCH, CH).transpose(0, 2, 1, 3))

    def prep_w(w):                                           # [H,D,dh] slice
        wc = np.ascontiguousarray(
            w[hs].transpose(1, 0, 2).reshape(D, HK))         # [D, HK]
        return _tile128(wc, (HK,)).astype(bfl)               # [128, DT, HK]

    wq, wk, wv = prep_w(W_Q), prep_w(W_K), prep_w(W_V)
    woc = W_O[hs].reshape(HK, D)                             # [HK, D]
    wo = _tile128(woc, (D,)).astype(bfl)                     # [128, 2, D]

    bq = np.ascontiguousarray(
        b_Q[hs].reshape(HK).reshape(2, 128).T).astype(np.float32)
    bk = np.ascontiguousarray(
        b_K[hs].reshape(HK).reshape(2, 128).T).astype(np.float32)

    # relative tril mask for one 128-key diagonal band
    r = np.arange(128)[:, None, None]
    f = np.arange(KB)[None, None, :]
    mask = np.repeat((f >= r), 2, axis=1).astype(bfl)        # [128, 2, 128]

    return {"xt": xt, "wq": wq, "wk": wk, "wv": wv, "wo": wo,
            "bq": bq, "bk": bk, "mask": mask}


def kernel(x, W_Q, W_K, W_V, W_O, b_Q, b_K, b_V, b_O, **run_kwargs):
    x = np.asarray(x, dtype=np.float32)
    W_Q = np.asarray(W_Q, dtype=np.float32)
    W_K = np.asarray(W_K, dtype=np.float32)
    W_V = np.asarray(W_V, dtype=np.float32)
    W_O = np.asarray(W_O, dtype=np.float32)
    b_Q = np.asarray(b_Q, dtype=np.float32)
    b_K = np.asarray(b_K, dtype=np.float32)
    b_V = np.asarray(b_V, dtype=np.float32)
    b_O = np.asarray(b_O, dtype=np.float32)

    if "nc" not in _CACHE:
        _CACHE["nc"] = _build_nc()
    nc = _CACHE["nc"]

    in_maps = []
    for i in range(NCORES):
        b, g = i // HG, i % HG
        in_maps.append(_prep_core(x, W_Q, W_K, W_V, W_O, b_Q, b_K, b, g))

    res = run_bass_kernel_spmd(nc, in_maps, core_ids=list(range(NCORES)),
                               **run_kwargs)

    # exact fold of b_V through W_O (softmax rows sum to 1), plus b_O
    bias = (b_O.astype(np.float64)
            + b_V.reshape(H * DH).astype(np.float64)
            @ W_O.reshape(H * DH, D).astype(np.float64)).astype(np.float32)

    out = np.zeros((B, S, D), dtype=np.float32)
    for i in range(NCORES):
        b = i // HG
        # outT layout [128(p), DT, S] -> [S, D] with d = dt*128 + p
        o = res.results[i]["outT"].astype(np.float32)
        out[b] += o.transpose(2, 1, 0).reshape(S, D)
    out += bias[None, None, :]
    if run_kwargs:
        return out, res
    return out
